# revision 17
# baseline (speedup 1.0000x reference)
"""Causal self-attention (B=4, T=2048, C=1024, H=16) on 8 TRN2 NeuronCores.

Sharding: 2D data-parallel (batch, 4-way) x tensor-parallel (heads, 2-way).
Core c handles batch b = c//2 and heads [8*(c%2), 8*(c%2)+8).

Per-core device program (all matmuls in float32r, N>=256 moving dim):
  1. Projections: qT/kT = W @ x^T in [O, T] layout, v in [T, O] layout
     (host pre-transposes x and the weight slices so no on-device
     transposes are needed).  v is stored interleaved per head with an
     appended ones column ("v_aug") so the attention row-sums fall out of
     the PE for free.
  2. Attention per head: att^T tiles [k,q] = K @ q^T, causal handled by
     restricting matmul column ranges + one triangular mask add on the
     diagonal 128-block; exp on ScalarE (scale fused); y^T = v_aug^T @
     att_exp accumulated in PSUM -> row 64 is the softmax denominator l.
     Normalize with 1/l broadcast across partitions.
  3. Output projection: y_part^T [C, T] = Wp_slice^T.T @ y^T.

Host gathers: y = sum of the two head-group partials per batch (+ bp),
present (k,v) reassembled from the per-core [O,T]/[T,O] projections.
"""

import sys

if "/opt/trn_rl_repo" not in sys.path:
    sys.path.insert(0, "/opt/trn_rl_repo")

import numpy as np

B, T_FULL, C, H, D = 4, 2048, 1024, 16, 64
NCORES = 8
HPC = H // 2  # heads per core (2-way tensor parallel)
O = HPC * D  # 512: per-core projection width
SCALE = 1.0 / np.sqrt(D)
NEG = -1.0e9

_PROG_CACHE = {}


def build_program(T=T_FULL):
    """Build the per-core Bass program (same SPMD program for all cores)."""
    from contextlib import ExitStack

    import concourse.bass as bass
    import concourse.mybir as mybir
    import concourse.tile as tile
    from concourse import bacc

    f32 = mybir.dt.float32
    f32r = mybir.dt.float32r
    ADD = mybir.AluOpType.add
    MUL = mybir.AluOpType.mult
    EXP = mybir.ActivationFunctionType.Exp

    assert T % 512 == 0
    NT = T // 512  # 512-wide t-chunks
    NTT = T // 128  # 128-wide t-tiles
    CC = C // 128  # contraction chunks over C

    nc = bacc.Bacc("TRN2", target_bir_lowering=False, debug=False)

    xT = nc.dram_tensor("xT", [C, T], f32, kind="ExternalInput").ap()
    wqT = nc.dram_tensor("wqT", [C, O], f32, kind="ExternalInput").ap()
    wkT = nc.dram_tensor("wkT", [C, O], f32, kind="ExternalInput").ap()
    wvT = nc.dram_tensor("wvT", [C, O], f32, kind="ExternalInput").ap()
    wpT = nc.dram_tensor("wpT", [O, C], f32, kind="ExternalInput").ap()
    bq = nc.dram_tensor("bq", [O], f32, kind="ExternalInput").ap()
    bk = nc.dram_tensor("bk", [O], f32, kind="ExternalInput").ap()
    bv = nc.dram_tensor("bv", [O], f32, kind="ExternalInput").ap()
    vones = nc.dram_tensor("vones", [128, HPC], f32, kind="ExternalInput").ap()
    kT_o = nc.dram_tensor("kT_out", [O, T], f32, kind="ExternalOutput").ap()
    v_o = nc.dram_tensor("v_out", [T, O], f32, kind="ExternalOutput").ap()
    yp_o = nc.dram_tensor("ypT", [C, T], f32, kind="ExternalOutput").ap()

    with tile.TileContext(nc) as tc, ExitStack() as ctx:
        # PSUM budget (8 banks): proj 2x1, att0 2, att1 2, yt 2x1.
        psum = ctx.enter_context(tc.tile_pool(name="psum", bufs=1, space="PSUM"))
        # Long-lived SBUF: qT/kT [O,T], v_aug, mask, bv broadcast.
        pk = ctx.enter_context(tc.tile_pool(name="pk", bufs=1))

        qT_sb = [pk.tile([128, T], f32r, name=f"qT{i}") for i in range(O // 128)]
        kT_sb = [pk.tile([128, T], f32r, name=f"kT{i}") for i in range(O // 128)]
        va_sb = [pk.tile([128, HPC, 65], f32r, name=f"va{i}") for i in range(NTT)]
        mask_sb = pk.tile([128, 128], f32, name="mask_sb")
        bvb = pk.tile([128, O], f32, name="bvb")

        # mask[k, q] = 0 where k <= q else NEG  (att^T layout: partition=k)
        nc.vector.memset(mask_sb, 0.0)
        nc.gpsimd.affine_select(
            out=mask_sb,
            in_=mask_sb,
            compare_op=mybir.AluOpType.is_ge,
            fill=NEG,
            base=0,
            pattern=[[1, 128]],
            channel_multiplier=-1,
        )
        nc.gpsimd.dma_start(
            out=bvb,
            in_=bass.AP(tensor=bv.tensor, offset=bv.offset, ap=[[0, 128], bv.ap[0]]),
        )

        # ---------------- Stage 1: q/k/v projections ----------------
        with (
            tc.tile_pool(name="pw", bufs=1) as pw,
            tc.tile_pool(name="px", bufs=1) as px,
        ):
            wq_sb = pw.tile([128, CC, O], f32r, name="wq_sb")
            wk_sb = pw.tile([128, CC, O], f32r, name="wk_sb")
            wv_sb = pw.tile([128, CC, O], f32r, name="wv_sb")
            bq_sb = pw.tile([128, O // 128], f32, name="bq_sb")
            bk_sb = pw.tile([128, O // 128], f32, name="bk_sb")
            def load_weights():
                # per C-chunk so the first matmuls can start early
                for ci in range(CC):
                    for wsb, wsrc in ((wq_sb, wqT), (wk_sb, wkT), (wv_sb, wvT)):
                        nc.sync.dma_start(
                            out=wsb[:, ci, :],
                            in_=wsrc.bitcast(f32r).rearrange("(c p) o -> p c o", p=128)[:, ci, :],
                        )
                nc.sync.dma_start(out=bq_sb, in_=bq.rearrange("(c p) -> p c", p=128))
                nc.sync.dma_start(out=bk_sb, in_=bk.rearrange("(c p) -> p c", p=128))

            for tci in range(NT):
                xc = []
                for ci in range(CC):
                    xt = px.tile([128, 512], f32r, name=f"xc{ci}", tag=f"xc{ci}", bufs=2)
                    nc.sync.dma_start(
                        out=xt, in_=xT.bitcast(f32r)[ci * 128 : (ci + 1) * 128, tci * 512 : (tci + 1) * 512]
                    )
                    xc.append(xt)
                if tci == 0:
                    load_weights()  # queued after the first x chunks
                # qT / kT chunks: out [o-chunk 128, t 512]
                for oi in range(O // 128):
                    for wsb, bsb, dst, dram in (
                        (wq_sb, bq_sb, qT_sb, None),
                        (wk_sb, bk_sb, kT_sb, kT_o),
                    ):
                        ps = psum.tile([128, 1024], f32, name="ps_proj", tag="big2", bufs=3)[:, 0:512]
                        for ci in range(CC):
                            nc.tensor.matmul(
                                ps,
                                wsb[:, ci, oi * 128 : (oi + 1) * 128],
                                xc[ci],
                                start=(ci == 0),
                                stop=(ci == CC - 1),
                            )
                        dsl = dst[oi][:, tci * 512 : (tci + 1) * 512]
                        nc.vector.tensor_scalar_add(dsl, ps, bsb[:, oi : oi + 1])
                        if dram is not None:
                            nc.sync.dma_start(
                                out=dram[oi * 128 : (oi + 1) * 128, tci * 512 : (tci + 1) * 512],
                                in_=dsl.bitcast(f32),
                            )
                # v chunks: out [t-tile 128, o 512]
                for j in range(4):
                    tt = tci * 4 + j
                    ps = psum.tile([128, 1024], f32, name="ps_proj", tag="big2", bufs=3)[:, 0:512]
                    for ci in range(CC):
                        nc.tensor.matmul(
                            ps,
                            xc[ci][:, j * 128 : (j + 1) * 128],
                            wv_sb[:, ci, :],
                            start=(ci == 0),
                            stop=(ci == CC - 1),
                        )
                    va = va_sb[tt]
                    nc.vector.tensor_tensor(
                        out=va[:, :, 0:64],
                        in0=ps.rearrange("p (h d) -> p h d", d=64),
                        in1=bvb.rearrange("p (h d) -> p h d", d=64),
                        op=ADD,
                    )
                    nc.sync.dma_start(
                        out=va[:, :, 64:65],
                        in_=vones.bitcast(f32r).rearrange("p (h o) -> p h o", o=1),
                    )
                    nc.sync.dma_start(
                        out=v_o[tt * 128 : (tt + 1) * 128, :].rearrange("p (h d) -> p h d", d=64),
                        in_=va[:, :, 0:64].bitcast(f32),
                    )

        # ---------------- Stages 2+3 ----------------
        with (
            tc.tile_pool(name="py", bufs=1) as py,
            tc.tile_pool(name="pa", bufs=1) as pa,
            tc.tile_pool(name="psm", bufs=1) as psm,
            tc.tile_pool(name="po", bufs=1) as po,
        ):
            wp_sb = py.tile([128, O // 128, C], f32r, name="wp_sb")
            nc.sync.dma_start(out=wp_sb, in_=wpT.bitcast(f32r).rearrange("(c p) n -> p c n", p=128))
            yT_all = [py.tile([128, T], f32r, name=f"yT{i}") for i in range(O // 128)]

            # Attention: head pair hp handles heads 2hp, 2hp+1 living in
            # partition halves [0,64) / [64,128) of qT_sb[hp]/kT_sb[hp].
            # qc-major so the output projection can start after the first
            # qc row completes for all head pairs.
            def emit_outproj(tcc):
                for co in range(CC):
                    ps = psum.tile([128, 1024], f32, name="ps_o", tag="big2", bufs=3)[:, 0:512]
                    for oi in range(O // 128):
                        nc.tensor.matmul(
                            ps,
                            wp_sb[:, oi, co * 128 : (co + 1) * 128],
                            yT_all[oi][:, tcc * 512 : (tcc + 1) * 512],
                            start=(oi == 0),
                            stop=(oi == O // 128 - 1),
                        )
                    osb = po.tile([128, 512], f32, name="osb", tag="osb", bufs=3)
                    nc.vector.tensor_copy(osb, ps)
                    nc.sync.dma_start(
                        out=yp_o[co * 128 : (co + 1) * 128, tcc * 512 : (tcc + 1) * 512],
                        in_=osb,
                    )

            for qc in range(NT):
                for hp in range(O // 128):
                    NK = 4 * (qc + 1)  # causal: key tiles 0..NK-1
                    yps = []
                    for hl in range(2):
                        yp = psum.tile([65, 512], f32, name=f"yt{hl}", tag="yt", bufs=2)
                        yps.append(yp)
                    for g in range(NK // 2):
                        aps2, axs2, qoffs = [], [], []
                        for hl in range(2):
                            aps2.append(
                                psum.tile([128, 1024], f32, name=f"att{hl}", tag="big2", bufs=3)
                            )
                            axs2.append(
                                pa.tile([128, 1024], f32r, name=f"ax{hl}", tag=f"ax{hl}", bufs=2)
                            )
                        # QK matmuls s-major: adjacent instructions alternate
                        # partition bases 0/64 -> row-group pairing on the PE
                        for s in range(2):
                            ki = 2 * g + s
                            qoff = max(0, ki * 128 - qc * 512)
                            qoffs.append(qoff)
                            for hl in range(2):
                                base = hl * 64
                                nc.tensor.matmul(
                                    aps2[hl][:, s * 512 + qoff : (s + 1) * 512],
                                    kT_sb[hp][base : base + 64, ki * 128 : (ki + 1) * 128],
                                    qT_sb[hp][base : base + 64, qc * 512 + qoff : (qc + 1) * 512],
                                    start=True,
                                    stop=True,
                                    tile_position=(base, 0),
                                )
                        for hl in range(2):
                            for s in range(2):
                                ki = 2 * g + s
                                if ki >= 4 * qc:  # diagonal tile: mask its 128-block
                                    sl = aps2[hl][:, s * 512 + qoffs[s] : s * 512 + qoffs[s] + 128]
                                    nc.vector.tensor_tensor(out=sl, in0=sl, in1=mask_sb, op=ADD)
                        for hl in range(2):
                            if qoffs == [0, 0]:
                                nc.scalar.activation(axs2[hl], aps2[hl], EXP, scale=float(SCALE))
                            else:
                                for s in range(2):
                                    r0 = s * 512 + qoffs[s]
                                    nc.scalar.activation(
                                        axs2[hl][:, r0 : (s + 1) * 512],
                                        aps2[hl][:, r0 : (s + 1) * 512],
                                        EXP,
                                        scale=float(SCALE),
                                    )
                        for hl in range(2):
                            h = 2 * hp + hl
                            for s in range(2):
                                ki = 2 * g + s
                                nc.tensor.matmul(
                                    yps[hl][:, qoffs[s] : 512],
                                    va_sb[ki][:, h, :],
                                    axs2[hl][:, s * 512 + qoffs[s] : (s + 1) * 512],
                                    start=(ki == 0),
                                    stop=(ki == NK - 1),
                                )
                    # normalize off the critical path: copy yt (y + l row) to
                    # SBUF, reciprocal in a [128, 4] spread layout (lane
                    # parallel), broadcast, multiply into yT_all.
                    for hl in range(2):
                        base = hl * 64
                        mir = psm.tile([65, 512], f32, name="mir", tag="mir", bufs=2)
                        nc.vector.tensor_copy(mir, yps[hl])
                        lfold = psm.tile([128, 4], f32, name="lfold", tag="lfold", bufs=2)
                        nc.sync.dma_start(out=lfold, in_=mir[64:65, :])
                        rfold = psm.tile([128, 4], f32, name="rfold", tag="rfold", bufs=2)
                        nc.vector.reciprocal(rfold, lfold)
                        rrow = psm.tile([1, 512], f32, name="rrow", tag="rrow", bufs=2)
                        nc.sync.dma_start(out=rrow, in_=rfold)
                        lbc = psm.tile([64, 512], f32, name="lbc", tag="lbc", bufs=2)
                        nc.gpsimd.partition_broadcast(lbc, rrow)
                        nc.vector.tensor_tensor(
                            out=yT_all[hp][base : base + 64, qc * 512 : (qc + 1) * 512],
                            in0=mir[0:64, :],
                            in1=lbc,
                            op=MUL,
                        )

                # ---- interleaved output projection, one row behind so the
                # PE never waits on the just-emitted normalize chain ----
                for tcc in ([qc - 1] if qc < NT - 1 else [qc - 1, qc]):
                    if tcc >= 0:
                        emit_outproj(tcc)

    nc.compile()
    return nc


def _get_program(T=T_FULL):
    if T not in _PROG_CACHE:
        _PROG_CACHE[T] = build_program(T)
    return _PROG_CACHE[T]


def make_in_maps(x, Wq, bq, Wk, bk, Wv, bv, Wp):
    """Shard full inputs into per-core input maps."""
    f = np.float32
    in_maps = []
    for c in range(NCORES):
        b, g = c // 2, c % 2
        sl = slice(g * O, (g + 1) * O)
        in_maps.append(
            {
                "xT": np.ascontiguousarray(np.asarray(x, f)[b].T),
                "wqT": np.ascontiguousarray(np.asarray(Wq, f)[sl].T),
                "wkT": np.ascontiguousarray(np.asarray(Wk, f)[sl].T),
                "wvT": np.ascontiguousarray(np.asarray(Wv, f)[sl].T),
                "wpT": np.ascontiguousarray(np.asarray(Wp, f)[:, sl].T),
                "bq": np.ascontiguousarray(np.asarray(bq, f)[sl]),
                "bk": np.ascontiguousarray(np.asarray(bk, f)[sl]),
                "bv": np.ascontiguousarray(np.asarray(bv, f)[sl]),
                "vones": np.ones((128, HPC), f),
            }
        )
    return in_maps


def gather_outputs(results, bp):
    """Assemble full (y, present) from per-core outputs."""
    f = np.float32
    T = results[0]["kT_out"].shape[1]
    y = np.zeros((B, T, C), f)
    k = np.empty((B, H, T, D), f)
    v = np.empty((B, H, T, D), f)
    for c in range(NCORES):
        b, g = c // 2, c % 2
        hs = slice(HPC * g, HPC * (g + 1))
        k[b, hs] = results[c]["kT_out"].reshape(HPC, D, T).transpose(0, 2, 1)
        v[b, hs] = results[c]["v_out"].reshape(T, HPC, D).transpose(1, 0, 2)
        y[b] += results[c]["ypT"].T
    y += np.asarray(bp, f)
    present = np.stack([k, v])
    return y, present


def kernel(x, Wq, bq, Wk, bk, Wv, bv, Wp, bp, _trace=False):
    from concourse.bass_utils import run_bass_kernel_spmd

    nc = _get_program(x.shape[1])
    in_maps = make_in_maps(x, Wq, bq, Wk, bk, Wv, bv, Wp)
    res = run_bass_kernel_spmd(nc, in_maps, core_ids=list(range(NCORES)), trace=_trace)
    out = gather_outputs(res.results, bp)
    if _trace:
        return out, res
    return out


# revision 21
# speedup vs baseline: 1.1946x; 1.1946x over previous
"""Causal self-attention (B=4, T=2048, C=1024, H=16) on 8 TRN2 NeuronCores.

Sharding: 2D data-parallel (batch, 4-way) x tensor-parallel (heads, 2-way).
Core c handles batch b = c//2 and heads [8*(c%2), 8*(c%2)+8).

Per-core device program (all matmuls in float32r, N>=256 moving dim):
  1. Projections: qT/kT = W @ x^T in [O, T] layout, v in [T, O] layout
     (host pre-transposes x and the weight slices so no on-device
     transposes are needed).  v is stored interleaved per head with an
     appended ones column ("v_aug") so the attention row-sums fall out of
     the PE for free.
  2. Attention per head: att^T tiles [k,q] = K @ q^T, causal handled by
     restricting matmul column ranges + one triangular mask add on the
     diagonal 128-block; exp on ScalarE (scale fused); y^T = v_aug^T @
     att_exp accumulated in PSUM -> row 64 is the softmax denominator l.
     Normalize with 1/l broadcast across partitions.
  3. Output projection: y_part^T [C, T] = Wp_slice^T.T @ y^T.

Host gathers: y = sum of the two head-group partials per batch (+ bp),
present (k,v) reassembled from the per-core [O,T]/[T,O] projections.
"""

import sys

if "/opt/trn_rl_repo" not in sys.path:
    sys.path.insert(0, "/opt/trn_rl_repo")

import numpy as np

B, T_FULL, C, H, D = 4, 2048, 1024, 16, 64
NCORES = 8
HPC = H // 2  # heads per core (2-way tensor parallel)
O = HPC * D  # 512: per-core projection width
SCALE = 1.0 / np.sqrt(D)
NEG = -1.0e9

# When True, the attention stage (QK logits, exp(att), att@V) runs in
# bfloat16; projections and the k/v/y outputs stay float32r-accurate.
ATT_BF16 = False

_PROG_CACHE = {}


def build_program(T=T_FULL, att_bf16=None):
    """Build the per-core Bass program (same SPMD program for all cores)."""
    from contextlib import ExitStack

    import concourse.bass as bass
    import concourse.mybir as mybir
    import concourse.tile as tile
    from concourse import bacc

    if att_bf16 is None:
        att_bf16 = ATT_BF16
    f32 = mybir.dt.float32
    f32r = mybir.dt.float32r
    adt = mybir.dt.bfloat16 if att_bf16 else f32r
    ADD = mybir.AluOpType.add
    MUL = mybir.AluOpType.mult
    EXP = mybir.ActivationFunctionType.Exp

    assert T % 512 == 0
    NT = T // 512  # 512-wide t-chunks
    NTT = T // 128  # 128-wide t-tiles
    CC = C // 128  # contraction chunks over C

    nc = bacc.Bacc("TRN2", target_bir_lowering=False, debug=False)

    xT = nc.dram_tensor("xT", [C, T], f32, kind="ExternalInput").ap()
    wqT = nc.dram_tensor("wqT", [C, O], f32, kind="ExternalInput").ap()
    wkT = nc.dram_tensor("wkT", [C, O], f32, kind="ExternalInput").ap()
    wvT = nc.dram_tensor("wvT", [C, O], f32, kind="ExternalInput").ap()
    wpT = nc.dram_tensor("wpT", [O, C], f32, kind="ExternalInput").ap()
    bq = nc.dram_tensor("bq", [O], f32, kind="ExternalInput").ap()
    bk = nc.dram_tensor("bk", [O], f32, kind="ExternalInput").ap()
    bv = nc.dram_tensor("bv", [O], f32, kind="ExternalInput").ap()
    vones = nc.dram_tensor("vones", [128, HPC], adt, kind="ExternalInput").ap()
    kT_o = nc.dram_tensor("kT_out", [O, T], f32, kind="ExternalOutput").ap()
    v_o = nc.dram_tensor("v_out", [T, O], f32, kind="ExternalOutput").ap()
    yp_o = nc.dram_tensor("ypT", [C, T], f32, kind="ExternalOutput").ap()

    with tile.TileContext(nc) as tc, ExitStack() as ctx:
        # PSUM budget (8 banks): proj 2x1, att0 2, att1 2, yt 2x1.
        psum = ctx.enter_context(tc.tile_pool(name="psum", bufs=1, space="PSUM"))
        # Long-lived SBUF: qT/kT [O,T], v_aug, mask, bv broadcast.
        pk = ctx.enter_context(tc.tile_pool(name="pk", bufs=1))

        qT_sb = [pk.tile([128, T], adt, name=f"qT{i}") for i in range(O // 128)]
        kT_sb = [pk.tile([128, T], adt, name=f"kT{i}") for i in range(O // 128)]
        va_sb = [pk.tile([128, HPC, 65], adt, name=f"va{i}") for i in range(NTT)]
        mask_sb = pk.tile([128, 128], f32, name="mask_sb")
        bvb = pk.tile([128, O], f32, name="bvb")

        # mask[k, q] = 0 where k <= q else NEG  (att^T layout: partition=k)
        nc.vector.memset(mask_sb, 0.0)
        nc.gpsimd.affine_select(
            out=mask_sb,
            in_=mask_sb,
            compare_op=mybir.AluOpType.is_ge,
            fill=NEG,
            base=0,
            pattern=[[1, 128]],
            channel_multiplier=-1,
        )
        nc.gpsimd.dma_start(
            out=bvb,
            in_=bass.AP(tensor=bv.tensor, offset=bv.offset, ap=[[0, 128], bv.ap[0]]),
        )

        # ---------------- Stage 1: q/k/v projections ----------------
        with (
            tc.tile_pool(name="pw", bufs=1) as pw,
            tc.tile_pool(name="px", bufs=1) as px,
        ):
            wq_sb = pw.tile([128, CC, O], f32r, name="wq_sb")
            wk_sb = pw.tile([128, CC, O], f32r, name="wk_sb")
            wv_sb = pw.tile([128, CC, O], f32r, name="wv_sb")
            bq_sb = pw.tile([128, O // 128], f32, name="bq_sb")
            bk_sb = pw.tile([128, O // 128], f32, name="bk_sb")
            nc.sync.dma_start(out=bq_sb, in_=bq.rearrange("(c p) -> p c", p=128))
            nc.sync.dma_start(out=bk_sb, in_=bk.rearrange("(c p) -> p c", p=128))

            for tci in range(NT):
                xc = []
                for ci in range(CC):
                    xt = px.tile([128, 512], f32r, name=f"xc{ci}", tag=f"xc{ci}", bufs=2)
                    nc.sync.dma_start(
                        out=xt, in_=xT.bitcast(f32r)[ci * 128 : (ci + 1) * 128, tci * 512 : (tci + 1) * 512]
                    )
                    xc.append(xt)
                    if tci == 0:
                        # interleave wq chunks with x chunks so the first q
                        # matmuls (need xc0 + wq0) start within ~1 us; wk/wv
                        # queue later, hidden under the q matmuls
                        nc.sync.dma_start(
                            out=wq_sb[:, ci, :],
                            in_=wqT.bitcast(f32r).rearrange("(c p) o -> p c o", p=128)[:, ci, :],
                        )
                # qT / kT chunks: out [o-chunk 128, t 512]
                for phase, (wsb, bsb, dst, dram) in enumerate(
                    (
                        (wq_sb, bq_sb, qT_sb, None),
                        (wk_sb, bk_sb, kT_sb, kT_o),
                    )
                ):
                    for oi in range(O // 128):
                        ps = psum.tile([128, 1024], f32, name="ps_proj", tag="big2", bufs=3)[:, 0:512]
                        for ci in range(CC):
                            nc.tensor.matmul(
                                ps,
                                wsb[:, ci, oi * 128 : (oi + 1) * 128],
                                xc[ci],
                                start=(ci == 0),
                                stop=(ci == CC - 1),
                            )
                        dsl = dst[oi][:, tci * 512 : (tci + 1) * 512]
                        nc.vector.tensor_scalar_add(dsl, ps, bsb[:, oi : oi + 1])
                        if dram is not None:
                            if att_bf16:
                                stg = px.tile([128, 512], f32, name="kstg", tag="kstg", bufs=3)
                                nc.vector.tensor_scalar_add(stg, ps, bsb[:, oi : oi + 1])
                                outsrc = stg
                            else:
                                outsrc = dsl.bitcast(f32)
                            nc.sync.dma_start(
                                out=dram[oi * 128 : (oi + 1) * 128, tci * 512 : (tci + 1) * 512],
                                in_=outsrc,
                            )
                    if tci == 0 and phase == 0:
                        for ci in range(CC):
                            nc.sync.dma_start(
                                out=wk_sb[:, ci, :],
                                in_=wkT.bitcast(f32r).rearrange("(c p) o -> p c o", p=128)[:, ci, :],
                            )
                        for ci in range(CC):
                            nc.sync.dma_start(
                                out=wv_sb[:, ci, :],
                                in_=wvT.bitcast(f32r).rearrange("(c p) o -> p c o", p=128)[:, ci, :],
                            )
                # v chunks: out [t-tile 128, o 512]
                for j in range(4):
                    tt = tci * 4 + j
                    ps = psum.tile([128, 1024], f32, name="ps_proj", tag="big2", bufs=3)[:, 0:512]
                    for ci in range(CC):
                        nc.tensor.matmul(
                            ps,
                            xc[ci][:, j * 128 : (j + 1) * 128],
                            wv_sb[:, ci, :],
                            start=(ci == 0),
                            stop=(ci == CC - 1),
                        )
                    va = va_sb[tt]
                    nc.vector.tensor_tensor(
                        out=va[:, :, 0:64],
                        in0=ps.rearrange("p (h d) -> p h d", d=64),
                        in1=bvb.rearrange("p (h d) -> p h d", d=64),
                        op=ADD,
                    )
                    nc.sync.dma_start(
                        out=va[:, :, 64:65],
                        in_=vones.rearrange("p (h o) -> p h o", o=1),
                    )
                    if att_bf16:
                        vstg = px.tile([128, 512], f32, name="vstg", tag="vstg", bufs=3)
                        nc.vector.tensor_tensor(out=vstg, in0=ps, in1=bvb, op=ADD)
                        vsrc = vstg.rearrange("p (h d) -> p h d", d=64)
                    else:
                        vsrc = va[:, :, 0:64].bitcast(f32)
                    nc.sync.dma_start(
                        out=v_o[tt * 128 : (tt + 1) * 128, :].rearrange("p (h d) -> p h d", d=64),
                        in_=vsrc,
                    )

        # ---------------- Stages 2+3 ----------------
        with (
            tc.tile_pool(name="py", bufs=1) as py,
            tc.tile_pool(name="pa", bufs=1) as pa,
            tc.tile_pool(name="psm", bufs=1) as psm,
            tc.tile_pool(name="po", bufs=1) as po,
        ):
            wp_sb = py.tile([128, O // 128, C], f32r, name="wp_sb")
            nc.sync.dma_start(out=wp_sb, in_=wpT.bitcast(f32r).rearrange("(c p) n -> p c n", p=128))
            yT_all = [py.tile([128, T], f32r, name=f"yT{i}") for i in range(O // 128)]

            # Attention: head pair hp handles heads 2hp, 2hp+1 living in
            # partition halves [0,64) / [64,128) of qT_sb[hp]/kT_sb[hp].
            # qc-major so the output projection can start after the first
            # qc row completes for all head pairs.
            def emit_outproj(tcc):
                for co in range(CC):
                    ps = psum.tile([128, 1024], f32, name="ps_o", tag="big2", bufs=3)[:, 0:512]
                    for oi in range(O // 128):
                        nc.tensor.matmul(
                            ps,
                            wp_sb[:, oi, co * 128 : (co + 1) * 128],
                            yT_all[oi][:, tcc * 512 : (tcc + 1) * 512],
                            start=(oi == 0),
                            stop=(oi == O // 128 - 1),
                        )
                    osb = po.tile([128, 512], f32, name="osb", tag="osb", bufs=3)
                    nc.vector.tensor_copy(osb, ps)
                    nc.sync.dma_start(
                        out=yp_o[co * 128 : (co + 1) * 128, tcc * 512 : (tcc + 1) * 512],
                        in_=osb,
                    )

            for qc in range(NT):
                for hp in range(O // 128):
                    NK = 4 * (qc + 1)  # causal: key tiles 0..NK-1
                    yps = []
                    for hl in range(2):
                        yp = psum.tile([65, 512], f32, name=f"yt{hl}", tag="yt", bufs=2)
                        yps.append(yp)
                    def emit_av(g, axs2, qoffs):
                        for hl in range(2):
                            h = 2 * hp + hl
                            for s in range(2):
                                ki = 2 * g + s
                                nc.tensor.matmul(
                                    yps[hl][:, qoffs[s] : 512],
                                    va_sb[ki][:, h, :],
                                    axs2[hl][:, s * 512 + qoffs[s] : (s + 1) * 512],
                                    start=(ki == 0),
                                    stop=(ki == NK - 1),
                                )

                    # software-pipelined: AV of group g-1 is emitted after
                    # QK+exp of group g, so the in-order PE never stalls on
                    # the exp of the group it just produced.
                    pend = None
                    for g in range(NK // 2):
                        aps2, axs2, qoffs = [], [], []
                        for hl in range(2):
                            aps2.append(
                                psum.tile([128, 1024], f32, name=f"att{hl}", tag="big2", bufs=3)
                            )
                            axs2.append(
                                pa.tile([128, 1024], adt, name=f"ax{hl}", tag=f"ax{hl}", bufs=3)
                            )
                        for s in range(2):
                            ki = 2 * g + s
                            qoff = max(0, ki * 128 - qc * 512)
                            qoffs.append(qoff)
                            for hl in range(2):
                                base = hl * 64
                                nc.tensor.matmul(
                                    aps2[hl][:, s * 512 + qoff : (s + 1) * 512],
                                    kT_sb[hp][base : base + 64, ki * 128 : (ki + 1) * 128],
                                    qT_sb[hp][base : base + 64, qc * 512 + qoff : (qc + 1) * 512],
                                    start=True,
                                    stop=True,
                                )
                        for hl in range(2):
                            for s in range(2):
                                ki = 2 * g + s
                                if ki >= 4 * qc:  # diagonal tile: mask its 128-block
                                    sl = aps2[hl][:, s * 512 + qoffs[s] : s * 512 + qoffs[s] + 128]
                                    nc.vector.tensor_tensor(out=sl, in0=sl, in1=mask_sb, op=ADD)
                        for hl in range(2):
                            if qoffs == [0, 0]:
                                nc.scalar.activation(axs2[hl], aps2[hl], EXP, scale=float(SCALE))
                            else:
                                for s in range(2):
                                    r0 = s * 512 + qoffs[s]
                                    nc.scalar.activation(
                                        axs2[hl][:, r0 : (s + 1) * 512],
                                        aps2[hl][:, r0 : (s + 1) * 512],
                                        EXP,
                                        scale=float(SCALE),
                                    )
                        if pend is not None:
                            emit_av(*pend)
                        pend = (g, axs2, qoffs)
                    emit_av(*pend)
                    # normalize off the critical path: copy yt (y + l row) to
                    # SBUF, reciprocal in a [128, 4] spread layout (lane
                    # parallel), broadcast, multiply into yT_all.
                    for hl in range(2):
                        base = hl * 64
                        mir = psm.tile([65, 512], f32, name="mir", tag="mir", bufs=2)
                        nc.vector.tensor_copy(mir, yps[hl])
                        lfold = psm.tile([128, 4], f32, name="lfold", tag="lfold", bufs=2)
                        nc.sync.dma_start(out=lfold, in_=mir[64:65, :])
                        rfold = psm.tile([128, 4], f32, name="rfold", tag="rfold", bufs=2)
                        nc.vector.reciprocal(rfold, lfold)
                        rrow = psm.tile([1, 512], f32, name="rrow", tag="rrow", bufs=2)
                        nc.sync.dma_start(out=rrow, in_=rfold)
                        lbc = psm.tile([64, 512], f32, name="lbc", tag="lbc", bufs=2)
                        nc.gpsimd.partition_broadcast(lbc, rrow)
                        nc.vector.tensor_tensor(
                            out=yT_all[hp][base : base + 64, qc * 512 : (qc + 1) * 512],
                            in0=mir[0:64, :],
                            in1=lbc,
                            op=MUL,
                        )

                # ---- interleaved output projection, one row behind so the
                # PE never waits on the just-emitted normalize chain ----
                for tcc in ([qc - 1] if qc < NT - 1 else [qc - 1, qc]):
                    if tcc >= 0:
                        emit_outproj(tcc)

    nc.compile()
    return nc


def _get_program(T=T_FULL):
    key = (T, ATT_BF16)
    if key not in _PROG_CACHE:
        _PROG_CACHE[key] = build_program(T, ATT_BF16)
    return _PROG_CACHE[key]


def make_in_maps(x, Wq, bq, Wk, bk, Wv, bv, Wp):
    """Shard full inputs into per-core input maps."""
    f = np.float32
    in_maps = []
    for c in range(NCORES):
        b, g = c // 2, c % 2
        sl = slice(g * O, (g + 1) * O)
        in_maps.append(
            {
                "xT": np.ascontiguousarray(np.asarray(x, f)[b].T),
                "wqT": np.ascontiguousarray(np.asarray(Wq, f)[sl].T),
                "wkT": np.ascontiguousarray(np.asarray(Wk, f)[sl].T),
                "wvT": np.ascontiguousarray(np.asarray(Wv, f)[sl].T),
                "wpT": np.ascontiguousarray(np.asarray(Wp, f)[:, sl].T),
                "bq": np.ascontiguousarray(np.asarray(bq, f)[sl]),
                "bk": np.ascontiguousarray(np.asarray(bk, f)[sl]),
                "bv": np.ascontiguousarray(np.asarray(bv, f)[sl]),
                "vones": np.ones(
                    (128, HPC),
                    __import__("ml_dtypes").bfloat16 if ATT_BF16 else f,
                ),
            }
        )
    return in_maps


def gather_outputs(results, bp):
    """Assemble full (y, present) from per-core outputs."""
    f = np.float32
    T = results[0]["kT_out"].shape[1]
    y = np.zeros((B, T, C), f)
    k = np.empty((B, H, T, D), f)
    v = np.empty((B, H, T, D), f)
    for c in range(NCORES):
        b, g = c // 2, c % 2
        hs = slice(HPC * g, HPC * (g + 1))
        k[b, hs] = results[c]["kT_out"].reshape(HPC, D, T).transpose(0, 2, 1)
        v[b, hs] = results[c]["v_out"].reshape(T, HPC, D).transpose(1, 0, 2)
        y[b] += results[c]["ypT"].T
    y += np.asarray(bp, f)
    present = np.stack([k, v])
    return y, present


def kernel(x, Wq, bq, Wk, bk, Wv, bv, Wp, bp, _trace=False):
    from concourse.bass_utils import run_bass_kernel_spmd

    nc = _get_program(x.shape[1])
    in_maps = make_in_maps(x, Wq, bq, Wk, bk, Wv, bv, Wp)
    res = run_bass_kernel_spmd(nc, in_maps, core_ids=list(range(NCORES)), trace=_trace)
    out = gather_outputs(res.results, bp)
    if _trace:
        return out, res
    return out
